# revision 37
# baseline (speedup 1.0000x reference)
"""CenterLoss-with-delta kernel for Trainium2 (8 NeuronCores, Bass/Tile).

Math (matches the reference):
    loss = 0.5 * sqrt(sum((deep_feat - centers[y])**2)) / B
    sums_c   = segment_sum(deep_feat, y)      counts_c = bincount(y)
    means_c  = sums_c / max(counts_c, 1)
    coeff_c  = counts_c / (1 + counts_c)      (coeff=0 kills absent classes,
                                               so the reference's rand_fill
                                               never reaches the output)
    grad_c   = coeff_c * (centers_c - means_c)
             = a_c * centers_c - b_c * sums_c,  a=cnt/(1+cnt), b=1/(1+cnt)

The gather term expands as
    sum||f - c_y||^2 = sum||f||^2 - 2*sum_c<sums_c, c_c> + sum_c cnt_c*||c_c||^2
so no per-row gather of centers is ever needed on device.

Distribution: rows are sorted by class on the host; core g owns classes
[125g, 125(g+1)) and exactly the rows labeled with them (padded with zero
rows to a static capacity). Class ownership is disjoint, so there are no
collectives: each core computes its 125 grad rows plus scalar loss partials,
and the host concatenates/sums.

On-device per core: one pass over its feature rows; a [128x128] one-hot
(iota == y) built on DVE feeds a TensorEngine matmul accumulating
sums[class, D] in PSUM over all row-tiles; ScalarEngine square-accumulates
sum||f||^2; DVE computes grad and the two loss reductions.
"""

import numpy as np

C = 1000          # num classes
D = 2048          # feature dim
B = 8192          # batch (only used for the final divide; actual rows taken
                  # from the input)
NCORES = 8
P = 128           # partitions
CLS_PER = C // NCORES   # 125 classes per core
NSLICE = D // 512       # PSUM-bank-sized matmul column slices

# Static row capacities (per core). Smallest one >= max shard size is used so
# the compiled NEFF is stable across typical (uniform) inputs.
_CAPS = (1152, 1280, 1536, 2048, 4096, 8192)

# Set by test harness to collect a profile; harmless when False.
PROFILE = False
PROFILE_DIR = None
LAST_RESULT = None

_build_cache = {}


def _build(nt):
    """Build + compile the per-core Bass program for nt row-tiles of 128."""
    import concourse.bacc as bacc
    import concourse.bass as bass
    import concourse.mybir as mybir
    import concourse.tile as tile

    f32 = mybir.dt.float32
    f16 = mybir.dt.float16
    Alu = mybir.AluOpType

    nc = bacc.Bacc("TRN2", target_bir_lowering=False, debug=False,
                   num_devices=NCORES)

    # Feat tiles are grouped into DMA chunks of up to 2 row-tiles (1MB fp16)
    # to cut dispatch + semaphore overhead on the sync queue. The first chunk
    # is a single tile so the PE pipeline starts one transfer earlier.
    chunks = [(0, 1)]
    t = 1
    while t < nt:
        c = min(2, nt - t)
        chunks.append((t, c))
        t += c
    nchunks = len(chunks)

    # feat travels as fp16 (host converts): halves the dominant DMA stream
    # and feeds the PE at 1 row/cycle. centers/grad stay f32 — grad accuracy
    # is set by centers and the fp32 PSUM accumulation of the one-hot matmul.
    feat_d = nc.dram_tensor("feat", [nt, P, D], f16, kind="ExternalInput")
    ycls_d = nc.dram_tensor("ycls", [P, nt], f32, kind="ExternalInput")
    iota_d = nc.dram_tensor("iota", [P, P], f32, kind="ExternalInput")
    cen_d = nc.dram_tensor("cen", [P, D], f32, kind="ExternalInput")
    ab_d = nc.dram_tensor("ab", [P, 2], f32, kind="ExternalInput")  # [a, -b]
    grad_d = nc.dram_tensor("grad", [P, D], f32, kind="ExternalOutput")
    # stats columns: [0, nt) = per-tile sum(f^2); nt = ||c_c||^2;
    # nt+1 = <sums_c, c_c>
    stats_d = nc.dram_tensor("stats", [P, nt + 2], f32,
                             kind="ExternalOutput")

    with tile.TileContext(nc) as tc:
        with (
            tc.tile_pool(name="const", bufs=1) as constp,
            tc.tile_pool(name="feat", bufs=3) as featp,
            tc.tile_pool(name="small", bufs=6) as smallp,
            tc.tile_pool(name="scratch", bufs=2) as scrp,
            tc.tile_pool(name="outs", bufs=1) as outp,
            tc.tile_pool(name="psum", bufs=1, space=bass.MemorySpace.PSUM) as psump,
        ):
            sums_ps = psump.tile([P, D], f32)     # 4 PSUM banks, [class, D]
            stats_t = outp.tile([P, nt + 2], f32)

            # DMA order: labels + iota (tiny) first, then feat chunks (PE
            # starts ASAP), end-of-kernel constants last.
            ys_t = constp.tile([P, nt], f32)
            nc.sync.dma_start(ys_t[:], ycls_d[:])
            iota_t = constp.tile([P, P], f32)
            nc.sync.dma_start(iota_t[:], iota_d[:])

            fcs = []
            for t0, c in chunks:
                fc = featp.tile([P, c, D], f16, tag="fc")
                nc.sync.dma_start(
                    fc[:], feat_d[t0:t0 + c].rearrange("t p d -> p t d"))
                fcs.append(fc)

            cen_t = constp.tile([P, D], f32)
            nc.sync.dma_start(cen_t[:], cen_d[:])
            ab_t = constp.tile([P, 2], f32)
            nc.sync.dma_start(ab_t[:], ab_d[:])

            # DVE does fp16 squares ~2x faster than ACT; 4 on ACT, 5 on DVE
            # balances the two engines against the ~12.6us DMA stream.
            act_sq = {0, 2, 4, 6}
            for ci, (t0, c) in enumerate(chunks):
                fc = fcs[ci]
                for s in range(c):
                    t = t0 + s
                    oh = smallp.tile([P, P], f16, tag="oh")
                    nc.vector.tensor_scalar(oh[:], iota_t[:],
                                            ys_t[:, t:t + 1], None,
                                            Alu.is_equal)
                    for n in range(NSLICE):
                        nc.tensor.matmul(
                            sums_ps[:, n * 512:(n + 1) * 512],
                            oh[:],
                            fc[:, s, n * 512:(n + 1) * 512],
                            start=(t == 0),
                            stop=(t == nt - 1),
                        )

                    sq_scr = scrp.tile([P, D], f16, tag="sqscr")
                    acc = stats_t[:, t:t + 1]
                    if t in act_sq:
                        nc.scalar.activation(
                            sq_scr[:], fc[:, s, :],
                            mybir.ActivationFunctionType.Square,
                            accum_out=acc)
                    else:
                        nc.vector.scalar_tensor_tensor(
                            sq_scr[:], fc[:, s, :], 1.0, fc[:, s, :],
                            Alu.mult, Alu.mult, accum_out=acc)

            # ||c_c||^2 and a*centers: only need centers, run under the loop
            junk_n = outp.tile([P, D], f32, tag="junk_n")
            nc.scalar.activation(junk_n[:], cen_t[:],
                                 mybir.ActivationFunctionType.Square,
                                 accum_out=stats_t[:, nt:nt + 1])
            t1 = outp.tile([P, D], f32, tag="t1")
            nc.scalar.mul(t1[:], cen_t[:], ab_t[:, 0:1])

            # Tail: per 512-column slice, grad = (-b)*sums + a*cen on DVE,
            # DMA it out immediately; the <sums_c, c_c> reduction follows.
            for n in range(NSLICE):
                sl = slice(n * 512, (n + 1) * 512)
                g_n = outp.tile([P, 512], f32, tag=f"grad{n}")
                nc.vector.scalar_tensor_tensor(g_n[:], sums_ps[:, sl],
                                               ab_t[:, 1:2], t1[:, sl],
                                               Alu.mult, Alu.add)
                nc.sync.dma_start(grad_d[:, sl], g_n[:])
            junk2 = outp.tile([P, D], f32, tag="junk2")
            nc.vector.scalar_tensor_tensor(
                junk2[:], sums_ps[:], 1.0, cen_t[:], Alu.mult, Alu.mult,
                accum_out=stats_t[:, nt + 1:nt + 2])
            nc.sync.dma_start(stats_d[:], stats_t[:])

    nc.compile()
    if not nc.is_finalized():
        nc.finalize()
    return nc


def _prepare(y, deep_feat, centers):
    """Host-side sharding: sort rows by class, build per-core input maps."""
    y = np.asarray(y)
    feat = np.ascontiguousarray(np.asarray(deep_feat), dtype=np.float32)
    cen = np.ascontiguousarray(np.asarray(centers), dtype=np.float32)

    yi = y.astype(np.int64)
    order = np.argsort(yi, kind="stable")
    ysorted = yi[order]
    cnt = np.bincount(ysorted, minlength=C).astype(np.int64)

    # Row range owned by each core (classes [125g, 125(g+1)))
    bounds = np.searchsorted(ysorted, np.arange(NCORES + 1) * CLS_PER)
    shard_rows = np.diff(bounds)
    cap = next((c for c in _CAPS if c >= shard_rows.max()), None)
    if cap is None:
        raise ValueError(f"shard of {shard_rows.max()} rows exceeds max capacity")
    nt = cap // P

    iota = np.broadcast_to(np.arange(P, dtype=np.float32), (P, P)).copy()

    in_maps = []
    host_cnt = []   # per-core padded local counts, for the loss combine
    for g in range(NCORES):
        lo, hi = bounds[g], bounds[g + 1]
        n_g = hi - lo

        feat_g = np.zeros((cap, D), dtype=np.float16)
        feat_g[:n_g] = feat[order[lo:hi]].astype(np.float16)

        ycls_g = np.full((cap,), CLS_PER + 1, dtype=np.float32)  # padding=126
        ycls_g[:n_g] = (ysorted[lo:hi] - g * CLS_PER).astype(np.float32)

        cen_g = np.zeros((P, D), dtype=np.float32)
        cen_g[:CLS_PER] = cen[g * CLS_PER:(g + 1) * CLS_PER]

        cnt_g = np.zeros((P,), dtype=np.float64)
        cnt_g[:CLS_PER] = cnt[g * CLS_PER:(g + 1) * CLS_PER]
        host_cnt.append(cnt_g)

        a_g = (cnt_g / (1.0 + cnt_g)).astype(np.float32)
        bn_g = np.where(cnt_g > 0, -1.0 / (1.0 + cnt_g), 0.0).astype(np.float32)

        in_maps.append({
            "feat": feat_g.reshape(nt, P, D),
            "ycls": np.ascontiguousarray(ycls_g.reshape(nt, P).T),
            "iota": iota,
            "cen": cen_g,
            "ab": np.stack([a_g, bn_g], axis=1),
        })

    return in_maps, host_cnt, nt


def _combine(results, host_cnt, nt, nrows):
    """Assemble full grad + loss from per-core outputs."""
    grad_full = np.concatenate(
        [results[g]["grad"][:CLS_PER] for g in range(NCORES)], axis=0)

    total = 0.0
    for g in range(NCORES):
        st = results[g]["stats"].astype(np.float64)
        sq = st[:, :nt].sum()
        normsq = st[:, nt]
        dotsc = st[:, nt + 1].sum()
        total += sq + (host_cnt[g] * normsq).sum() - 2.0 * dotsc

    loss = np.float32(0.5 * np.sqrt(total) / nrows)
    return loss, grad_full


def kernel(y, deep_feat, centers):
    global LAST_RESULT
    from concourse.bass_utils import run_bass_kernel_spmd

    nrows = np.asarray(deep_feat).shape[0]
    in_maps, host_cnt, nt = _prepare(y, deep_feat, centers)

    if nt not in _build_cache:
        _build_cache[nt] = _build(nt)
    nc = _build_cache[nt]

    res = run_bass_kernel_spmd(nc, in_maps, list(range(NCORES)),
                               trace=PROFILE, tmpdir=PROFILE_DIR)
    LAST_RESULT = res
    return _combine(res.results, host_cnt, nt, nrows)


# revision 40
# speedup vs baseline: 1.0457x; 1.0457x over previous
"""CenterLoss-with-delta kernel for Trainium2 (8 NeuronCores, Bass/Tile).

Math (matches the reference):
    loss = 0.5 * sqrt(sum((deep_feat - centers[y])**2)) / B
    sums_c   = segment_sum(deep_feat, y)      counts_c = bincount(y)
    means_c  = sums_c / max(counts_c, 1)
    coeff_c  = counts_c / (1 + counts_c)      (coeff=0 kills absent classes,
                                               so the reference's rand_fill
                                               never reaches the output)
    grad_c   = coeff_c * (centers_c - means_c)
             = a_c * centers_c - b_c * sums_c,  a=cnt/(1+cnt), b=1/(1+cnt)

The gather term expands as
    sum||f - c_y||^2 = sum||f||^2 - 2*sum_c<sums_c, c_c> + sum_c cnt_c*||c_c||^2
so no per-row gather of centers is ever needed on device.

Distribution: rows are sorted by class on the host; core g owns classes
[125g, 125(g+1)) and exactly the rows labeled with them (padded with zero
rows to a static capacity). Class ownership is disjoint, so there are no
collectives: each core computes its 125 grad rows plus scalar loss partials,
and the host concatenates/sums.

On-device per core: one pass over its feature rows; a [128x128] one-hot
(iota == y) built on DVE feeds a TensorEngine matmul accumulating
sums[class, D] in PSUM over all row-tiles; ScalarEngine square-accumulates
sum||f||^2; DVE computes grad and the two loss reductions.
"""

import numpy as np

C = 1000          # num classes
D = 2048          # feature dim
B = 8192          # batch (only used for the final divide; actual rows taken
                  # from the input)
NCORES = 8
P = 128           # partitions
CLS_PER = C // NCORES   # 125 classes per core
NSLICE = D // 512       # PSUM-bank-sized matmul column slices

# Static row capacities (per core). Smallest one >= max shard size is used so
# the compiled NEFF is stable across typical (uniform) inputs.
_CAPS = (1152, 1280, 1536, 2048, 4096, 8192)

# Set by test harness to collect a profile; harmless when False.
PROFILE = False
PROFILE_DIR = None
LAST_RESULT = None

_build_cache = {}


def _build(nt):
    """Build + compile the per-core Bass program for nt row-tiles of 128."""
    import concourse.bacc as bacc
    import concourse.bass as bass
    import concourse.mybir as mybir
    import concourse.tile as tile

    f32 = mybir.dt.float32
    f16 = mybir.dt.float16
    Alu = mybir.AluOpType

    nc = bacc.Bacc("TRN2", target_bir_lowering=False, debug=False,
                   num_devices=NCORES)

    # Feat tiles are grouped into DMA chunks of up to 2 row-tiles (1MB fp16)
    # to cut dispatch + semaphore overhead on the sync queue. The first chunk
    # is a single tile so the PE pipeline starts one transfer earlier.
    chunks = [(0, 1)]
    t = 1
    while t < nt:
        c = min(2, nt - t)
        chunks.append((t, c))
        t += c
    nchunks = len(chunks)

    # feat travels as fp16 (host converts): halves the dominant DMA stream
    # and feeds the PE at 1 row/cycle. centers/grad stay f32 — grad accuracy
    # is set by centers and the fp32 PSUM accumulation of the one-hot matmul.
    feat_d = nc.dram_tensor("feat", [nt, P, D], f16, kind="ExternalInput")
    ycls_d = nc.dram_tensor("ycls", [P, nt], f32, kind="ExternalInput")
    iota_d = nc.dram_tensor("iota", [P, P], f32, kind="ExternalInput")
    cen_d = nc.dram_tensor("cen", [P, D], f32, kind="ExternalInput")
    ab_d = nc.dram_tensor("ab", [P, 2], f32, kind="ExternalInput")  # [a, -b]
    grad_d = nc.dram_tensor("grad", [P, D], f32, kind="ExternalOutput")
    # stats columns: [0, nt) = per-tile sum(f^2); nt = ||c_c||^2;
    # nt+1 = <sums_c, c_c>
    stats_d = nc.dram_tensor("stats", [P, nt + 2], f32,
                             kind="ExternalOutput")

    with tile.TileContext(nc) as tc:
        with (
            tc.tile_pool(name="const", bufs=1) as constp,
            tc.tile_pool(name="feat", bufs=3) as featp,
            tc.tile_pool(name="small", bufs=6) as smallp,
            tc.tile_pool(name="scratch", bufs=2) as scrp,
            tc.tile_pool(name="outs", bufs=1) as outp,
            tc.tile_pool(name="psum", bufs=1, space=bass.MemorySpace.PSUM) as psump,
        ):
            sums_ps = psump.tile([P, D], f32)     # 4 PSUM banks, [class, D]
            stats_t = outp.tile([P, nt + 2], f32)

            # DMA order: labels + iota (tiny) first, then feat chunks (PE
            # starts ASAP), end-of-kernel constants last.
            ys_t = constp.tile([P, nt], f32)
            nc.sync.dma_start(ys_t[:], ycls_d[:])
            iota_t = constp.tile([P, P], f32)
            nc.sync.dma_start(iota_t[:], iota_d[:])

            fcs = []
            cen_t = constp.tile([P, D], f32)
            ab_t = constp.tile([P, 2], f32)
            for ci, (t0, c) in enumerate(chunks):
                fc = featp.tile([P, c, D], f16, tag="fc")
                nc.sync.dma_start(
                    fc[:], feat_d[t0:t0 + c].rearrange("t p d -> p t d"))
                fcs.append(fc)
                if ci == 1:
                    # centers + coeffs after the first two feat chunks: early
                    # enough for ScalarE to run t1/normsq during the stream,
                    # late enough not to delay the PE ramp.
                    nc.sync.dma_start(cen_t[:], cen_d[:])
                    nc.sync.dma_start(ab_t[:], ab_d[:])

            # Alternate squares ScalarE/VectorE so they run concurrently;
            # ACT takes the last tile so DVE is free when the tail starts.
            act_sq = {0, 2, 4, 6, 8, 10, 12, 14, 16}
            t1 = outp.tile([P, D], f32, tag="t1")
            junk_n = outp.tile([P, D], f32, tag="junk_n")
            for ci, (t0, c) in enumerate(chunks):
                if ci == 2:
                    # a*centers and ||c_c||^2 hoisted into ScalarE's idle
                    # window mid-stream; the tail's grad slices need t1.
                    nc.scalar.mul(t1[:], cen_t[:], ab_t[:, 0:1])
                    nc.scalar.activation(
                        junk_n[:], cen_t[:],
                        mybir.ActivationFunctionType.Square,
                        accum_out=stats_t[:, nt:nt + 1])
                fc = fcs[ci]
                for s in range(c):
                    t = t0 + s
                    oh = smallp.tile([P, P], f16, tag="oh")
                    nc.vector.tensor_scalar(oh[:], iota_t[:],
                                            ys_t[:, t:t + 1], None,
                                            Alu.is_equal)
                    for n in range(NSLICE):
                        nc.tensor.matmul(
                            sums_ps[:, n * 512:(n + 1) * 512],
                            oh[:],
                            fc[:, s, n * 512:(n + 1) * 512],
                            start=(t == 0),
                            stop=(t == nt - 1),
                        )

                    sq_scr = scrp.tile([P, D], f16, tag="sqscr")
                    acc = stats_t[:, t:t + 1]
                    if t in act_sq:
                        nc.scalar.activation(
                            sq_scr[:], fc[:, s, :],
                            mybir.ActivationFunctionType.Square,
                            accum_out=acc)
                    else:
                        nc.vector.scalar_tensor_tensor(
                            sq_scr[:], fc[:, s, :], 1.0, fc[:, s, :],
                            Alu.mult, Alu.mult, accum_out=acc)

            # Tail: per 512-column slice, grad = (-b)*sums + a*cen on DVE,
            # DMA it out immediately; the <sums_c, c_c> reduction follows.
            for n in range(NSLICE):
                sl = slice(n * 512, (n + 1) * 512)
                g_n = outp.tile([P, 512], f32, tag=f"grad{n}")
                nc.vector.scalar_tensor_tensor(g_n[:], sums_ps[:, sl],
                                               ab_t[:, 1:2], t1[:, sl],
                                               Alu.mult, Alu.add)
                nc.sync.dma_start(grad_d[:, sl], g_n[:])
            junk2 = outp.tile([P, D], f32, tag="junk2")
            nc.vector.scalar_tensor_tensor(
                junk2[:], sums_ps[:], 1.0, cen_t[:], Alu.mult, Alu.mult,
                accum_out=stats_t[:, nt + 1:nt + 2])
            nc.sync.dma_start(stats_d[:], stats_t[:])

    nc.compile()
    if not nc.is_finalized():
        nc.finalize()
    return nc


def _prepare(y, deep_feat, centers):
    """Host-side sharding: sort rows by class, build per-core input maps."""
    y = np.asarray(y)
    feat = np.ascontiguousarray(np.asarray(deep_feat), dtype=np.float32)
    cen = np.ascontiguousarray(np.asarray(centers), dtype=np.float32)

    yi = y.astype(np.int64)
    order = np.argsort(yi, kind="stable")
    ysorted = yi[order]
    cnt = np.bincount(ysorted, minlength=C).astype(np.int64)

    # Row range owned by each core (classes [125g, 125(g+1)))
    bounds = np.searchsorted(ysorted, np.arange(NCORES + 1) * CLS_PER)
    shard_rows = np.diff(bounds)
    cap = next((c for c in _CAPS if c >= shard_rows.max()), None)
    if cap is None:
        raise ValueError(f"shard of {shard_rows.max()} rows exceeds max capacity")
    nt = cap // P

    iota = np.broadcast_to(np.arange(P, dtype=np.float32), (P, P)).copy()

    in_maps = []
    host_cnt = []   # per-core padded local counts, for the loss combine
    for g in range(NCORES):
        lo, hi = bounds[g], bounds[g + 1]
        n_g = hi - lo

        feat_g = np.zeros((cap, D), dtype=np.float16)
        feat_g[:n_g] = feat[order[lo:hi]].astype(np.float16)

        ycls_g = np.full((cap,), CLS_PER + 1, dtype=np.float32)  # padding=126
        ycls_g[:n_g] = (ysorted[lo:hi] - g * CLS_PER).astype(np.float32)

        cen_g = np.zeros((P, D), dtype=np.float32)
        cen_g[:CLS_PER] = cen[g * CLS_PER:(g + 1) * CLS_PER]

        cnt_g = np.zeros((P,), dtype=np.float64)
        cnt_g[:CLS_PER] = cnt[g * CLS_PER:(g + 1) * CLS_PER]
        host_cnt.append(cnt_g)

        a_g = (cnt_g / (1.0 + cnt_g)).astype(np.float32)
        bn_g = np.where(cnt_g > 0, -1.0 / (1.0 + cnt_g), 0.0).astype(np.float32)

        in_maps.append({
            "feat": feat_g.reshape(nt, P, D),
            "ycls": np.ascontiguousarray(ycls_g.reshape(nt, P).T),
            "iota": iota,
            "cen": cen_g,
            "ab": np.stack([a_g, bn_g], axis=1),
        })

    return in_maps, host_cnt, nt


def _combine(results, host_cnt, nt, nrows):
    """Assemble full grad + loss from per-core outputs."""
    grad_full = np.concatenate(
        [results[g]["grad"][:CLS_PER] for g in range(NCORES)], axis=0)

    total = 0.0
    for g in range(NCORES):
        st = results[g]["stats"].astype(np.float64)
        sq = st[:, :nt].sum()
        normsq = st[:, nt]
        dotsc = st[:, nt + 1].sum()
        total += sq + (host_cnt[g] * normsq).sum() - 2.0 * dotsc

    loss = np.float32(0.5 * np.sqrt(total) / nrows)
    return loss, grad_full


def kernel(y, deep_feat, centers):
    global LAST_RESULT
    from concourse.bass_utils import run_bass_kernel_spmd

    nrows = np.asarray(deep_feat).shape[0]
    in_maps, host_cnt, nt = _prepare(y, deep_feat, centers)

    if nt not in _build_cache:
        _build_cache[nt] = _build(nt)
    nc = _build_cache[nt]

    res = run_bass_kernel_spmd(nc, in_maps, list(range(NCORES)),
                               trace=PROFILE, tmpdir=PROFILE_DIR)
    LAST_RESULT = res
    return _combine(res.results, host_cnt, nt, nrows)


# revision 42
# speedup vs baseline: 1.0887x; 1.0411x over previous
"""CenterLoss-with-delta kernel for Trainium2 (8 NeuronCores, Bass/Tile).

Math (matches the reference):
    loss = 0.5 * sqrt(sum((deep_feat - centers[y])**2)) / B
    sums_c   = segment_sum(deep_feat, y)      counts_c = bincount(y)
    means_c  = sums_c / max(counts_c, 1)
    coeff_c  = counts_c / (1 + counts_c)      (coeff=0 kills absent classes,
                                               so the reference's rand_fill
                                               never reaches the output)
    grad_c   = coeff_c * (centers_c - means_c)
             = a_c * centers_c - b_c * sums_c,  a=cnt/(1+cnt), b=1/(1+cnt)

The gather term expands as
    sum||f - c_y||^2 = sum||f||^2 - 2*sum_c<sums_c, c_c> + sum_c cnt_c*||c_c||^2
so no per-row gather of centers is ever needed on device.

Distribution: rows are sorted by class on the host; core g owns classes
[125g, 125(g+1)) and exactly the rows labeled with them (padded with zero
rows to a static capacity). Class ownership is disjoint, so there are no
collectives: each core computes its 125 grad rows plus scalar loss partials,
and the host concatenates/sums.

On-device per core: one pass over its feature rows; a [128x128] one-hot
(iota == y) built on DVE feeds a TensorEngine matmul accumulating
sums[class, D] in PSUM over all row-tiles; ScalarEngine square-accumulates
sum||f||^2; DVE computes grad and the two loss reductions.
"""

import numpy as np

C = 1000          # num classes
D = 2048          # feature dim
B = 8192          # batch (only used for the final divide; actual rows taken
                  # from the input)
NCORES = 8
P = 128           # partitions
CLS_PER = C // NCORES   # 125 classes per core
NSLICE = D // 512       # PSUM-bank-sized matmul column slices

# Static row capacities (per core). Smallest one >= max shard size is used so
# the compiled NEFF is stable across typical (uniform) inputs.
_CAPS = (1152, 1280, 1536, 2048, 4096, 8192)

# Set by test harness to collect a profile; harmless when False.
PROFILE = False
PROFILE_DIR = None
LAST_RESULT = None

_build_cache = {}


def _build(nt):
    """Build + compile the per-core Bass program for nt row-tiles of 128."""
    import concourse.bacc as bacc
    import concourse.bass as bass
    import concourse.mybir as mybir
    import concourse.tile as tile

    f32 = mybir.dt.float32
    f16 = mybir.dt.float16
    Alu = mybir.AluOpType

    nc = bacc.Bacc("TRN2", target_bir_lowering=False, debug=False,
                   num_devices=NCORES)

    # Feat tiles are grouped into DMA chunks of up to 2 row-tiles (1MB fp16)
    # to cut dispatch + semaphore overhead on the sync queue. The first chunk
    # is a single tile so the PE pipeline starts one transfer earlier.
    chunks = [(0, 1)]
    t = 1
    while t < nt:
        c = min(2, nt - t)
        chunks.append((t, c))
        t += c
    nchunks = len(chunks)

    # feat travels as fp16 (host converts): halves the dominant DMA stream
    # and feeds the PE at 1 row/cycle. centers/grad stay f32 — grad accuracy
    # is set by centers and the fp32 PSUM accumulation of the one-hot matmul.
    feat_d = nc.dram_tensor("feat", [nt, P, D], f16, kind="ExternalInput")
    ycls_d = nc.dram_tensor("ycls", [P, nt], f32, kind="ExternalInput")
    iota_d = nc.dram_tensor("iota", [P, P], f32, kind="ExternalInput")
    cen_d = nc.dram_tensor("cen", [P, D], f32, kind="ExternalInput")
    ab_d = nc.dram_tensor("ab", [P, 2], f32, kind="ExternalInput")  # [a, -b]
    grad_d = nc.dram_tensor("grad", [P, D], f32, kind="ExternalOutput")
    # stats columns: [0, nt) = per-tile sum(f^2); nt = ||c_c||^2;
    # nt+1 = <sums_c, c_c>
    stats_d = nc.dram_tensor("stats", [P, nt + 2], f32,
                             kind="ExternalOutput")

    with tile.TileContext(nc) as tc:
        with (
            tc.tile_pool(name="const", bufs=1) as constp,
            tc.tile_pool(name="feat", bufs=4) as featp,
            tc.tile_pool(name="small", bufs=6) as smallp,
            tc.tile_pool(name="scratch", bufs=2) as scrp,
            tc.tile_pool(name="outs", bufs=1) as outp,
            tc.tile_pool(name="psum", bufs=1, space=bass.MemorySpace.PSUM) as psump,
        ):
            sums_ps = psump.tile([P, D], f32)     # 4 PSUM banks, [class, D]
            stats_t = outp.tile([P, nt + 2], f32)

            # DMA order: labels + iota (tiny) first, then feat chunks (PE
            # starts ASAP), end-of-kernel constants last.
            ys_t = constp.tile([P, nt], f32)
            nc.sync.dma_start(ys_t[:], ycls_d[:])
            iota_t = constp.tile([P, P], f32)
            nc.sync.dma_start(iota_t[:], iota_d[:])

            fcs = []
            cen_t = constp.tile([P, D], f32)
            ab_t = constp.tile([P, 2], f32)
            for ci, (t0, c) in enumerate(chunks):
                fc = featp.tile([P, c, D], f16, tag="fc")
                nc.sync.dma_start(
                    fc[:], feat_d[t0:t0 + c].rearrange("t p d -> p t d"))
                fcs.append(fc)
                if ci == 3:
                    # centers + coeffs late in the feat stream: early enough
                    # for ScalarE to run t1/normsq before the tail, late
                    # enough not to starve the PE while it is still cold.
                    nc.sync.dma_start(cen_t[:], cen_d[:])
                    nc.sync.dma_start(ab_t[:], ab_d[:])

            # Alternate squares ScalarE/VectorE so they run concurrently;
            # ACT takes the last tile so DVE is free when the tail starts.
            act_sq = {0, 2, 4, 6, 8, 10, 12, 14, 16}
            t1 = outp.tile([P, D], f32, tag="t1")
            junk_n = outp.tile([P, D], f32, tag="junk_n")
            for ci, (t0, c) in enumerate(chunks):
                if ci == 4:
                    # a*centers and ||c_c||^2 hoisted into ScalarE's idle
                    # window mid-stream; the tail's grad slices need t1.
                    nc.scalar.mul(t1[:], cen_t[:], ab_t[:, 0:1])
                    nc.scalar.activation(
                        junk_n[:], cen_t[:],
                        mybir.ActivationFunctionType.Square,
                        accum_out=stats_t[:, nt:nt + 1])
                fc = fcs[ci]
                for s in range(c):
                    t = t0 + s
                    oh = smallp.tile([P, P], f16, tag="oh")
                    nc.vector.tensor_scalar(oh[:], iota_t[:],
                                            ys_t[:, t:t + 1], None,
                                            Alu.is_equal)
                    for n in range(NSLICE):
                        nc.tensor.matmul(
                            sums_ps[:, n * 512:(n + 1) * 512],
                            oh[:],
                            fc[:, s, n * 512:(n + 1) * 512],
                            start=(t == 0),
                            stop=(t == nt - 1),
                        )

                    sq_scr = scrp.tile([P, D], f16, tag="sqscr")
                    acc = stats_t[:, t:t + 1]
                    if t in act_sq:
                        nc.scalar.activation(
                            sq_scr[:], fc[:, s, :],
                            mybir.ActivationFunctionType.Square,
                            accum_out=acc)
                    else:
                        nc.vector.scalar_tensor_tensor(
                            sq_scr[:], fc[:, s, :], 1.0, fc[:, s, :],
                            Alu.mult, Alu.mult, accum_out=acc)

            # Tail: per 512-column slice, grad = (-b)*sums + a*cen on DVE,
            # DMA it out immediately; the <sums_c, c_c> reduction follows.
            for n in range(NSLICE):
                sl = slice(n * 512, (n + 1) * 512)
                g_n = outp.tile([P, 512], f32, tag=f"grad{n}")
                nc.vector.scalar_tensor_tensor(g_n[:], sums_ps[:, sl],
                                               ab_t[:, 1:2], t1[:, sl],
                                               Alu.mult, Alu.add)
                nc.sync.dma_start(grad_d[:, sl], g_n[:])
            junk2 = outp.tile([P, D], f32, tag="junk2")
            nc.vector.scalar_tensor_tensor(
                junk2[:], sums_ps[:], 1.0, cen_t[:], Alu.mult, Alu.mult,
                accum_out=stats_t[:, nt + 1:nt + 2])
            nc.sync.dma_start(stats_d[:], stats_t[:])

    nc.compile()
    if not nc.is_finalized():
        nc.finalize()
    return nc


def _prepare(y, deep_feat, centers):
    """Host-side sharding: sort rows by class, build per-core input maps."""
    y = np.asarray(y)
    feat = np.ascontiguousarray(np.asarray(deep_feat), dtype=np.float32)
    cen = np.ascontiguousarray(np.asarray(centers), dtype=np.float32)

    yi = y.astype(np.int64)
    order = np.argsort(yi, kind="stable")
    ysorted = yi[order]
    cnt = np.bincount(ysorted, minlength=C).astype(np.int64)

    # Row range owned by each core (classes [125g, 125(g+1)))
    bounds = np.searchsorted(ysorted, np.arange(NCORES + 1) * CLS_PER)
    shard_rows = np.diff(bounds)
    cap = next((c for c in _CAPS if c >= shard_rows.max()), None)
    if cap is None:
        raise ValueError(f"shard of {shard_rows.max()} rows exceeds max capacity")
    nt = cap // P

    iota = np.broadcast_to(np.arange(P, dtype=np.float32), (P, P)).copy()

    in_maps = []
    host_cnt = []   # per-core padded local counts, for the loss combine
    for g in range(NCORES):
        lo, hi = bounds[g], bounds[g + 1]
        n_g = hi - lo

        feat_g = np.zeros((cap, D), dtype=np.float16)
        feat_g[:n_g] = feat[order[lo:hi]].astype(np.float16)

        ycls_g = np.full((cap,), CLS_PER + 1, dtype=np.float32)  # padding=126
        ycls_g[:n_g] = (ysorted[lo:hi] - g * CLS_PER).astype(np.float32)

        cen_g = np.zeros((P, D), dtype=np.float32)
        cen_g[:CLS_PER] = cen[g * CLS_PER:(g + 1) * CLS_PER]

        cnt_g = np.zeros((P,), dtype=np.float64)
        cnt_g[:CLS_PER] = cnt[g * CLS_PER:(g + 1) * CLS_PER]
        host_cnt.append(cnt_g)

        a_g = (cnt_g / (1.0 + cnt_g)).astype(np.float32)
        bn_g = np.where(cnt_g > 0, -1.0 / (1.0 + cnt_g), 0.0).astype(np.float32)

        in_maps.append({
            "feat": feat_g.reshape(nt, P, D),
            "ycls": np.ascontiguousarray(ycls_g.reshape(nt, P).T),
            "iota": iota,
            "cen": cen_g,
            "ab": np.stack([a_g, bn_g], axis=1),
        })

    return in_maps, host_cnt, nt


def _combine(results, host_cnt, nt, nrows):
    """Assemble full grad + loss from per-core outputs."""
    grad_full = np.concatenate(
        [results[g]["grad"][:CLS_PER] for g in range(NCORES)], axis=0)

    total = 0.0
    for g in range(NCORES):
        st = results[g]["stats"].astype(np.float64)
        sq = st[:, :nt].sum()
        normsq = st[:, nt]
        dotsc = st[:, nt + 1].sum()
        total += sq + (host_cnt[g] * normsq).sum() - 2.0 * dotsc

    loss = np.float32(0.5 * np.sqrt(total) / nrows)
    return loss, grad_full


def kernel(y, deep_feat, centers):
    global LAST_RESULT
    from concourse.bass_utils import run_bass_kernel_spmd

    nrows = np.asarray(deep_feat).shape[0]
    in_maps, host_cnt, nt = _prepare(y, deep_feat, centers)

    if nt not in _build_cache:
        _build_cache[nt] = _build(nt)
    nc = _build_cache[nt]

    res = run_bass_kernel_spmd(nc, in_maps, list(range(NCORES)),
                               trace=PROFILE, tmpdir=PROFILE_DIR)
    LAST_RESULT = res
    return _combine(res.results, host_cnt, nt, nrows)


# revision 46
# speedup vs baseline: 1.1155x; 1.0245x over previous
"""CenterLoss-with-delta kernel for Trainium2 (8 NeuronCores, Bass/Tile).

Math (matches the reference):
    loss = 0.5 * sqrt(sum((deep_feat - centers[y])**2)) / B
    sums_c   = segment_sum(deep_feat, y)      counts_c = bincount(y)
    means_c  = sums_c / max(counts_c, 1)
    coeff_c  = counts_c / (1 + counts_c)      (coeff=0 kills absent classes,
                                               so the reference's rand_fill
                                               never reaches the output)
    grad_c   = coeff_c * (centers_c - means_c)
             = a_c * centers_c - b_c * sums_c,  a=cnt/(1+cnt), b=1/(1+cnt)

The gather term expands as
    sum||f - c_y||^2 = sum||f||^2 - 2*sum_c<sums_c, c_c> + sum_c cnt_c*||c_c||^2
so no per-row gather of centers is ever needed on device.

Distribution: rows are sorted by class on the host; core g owns classes
[125g, 125(g+1)) and exactly the rows labeled with them (padded with zero
rows to a static capacity). Class ownership is disjoint, so there are no
collectives: each core computes its 125 grad rows plus scalar loss partials,
and the host concatenates/sums.

On-device per core: one pass over its feature rows; a [128x128] one-hot
(iota == y) built on DVE feeds a TensorEngine matmul accumulating
sums[class, D] in PSUM over all row-tiles; ScalarEngine square-accumulates
sum||f||^2; DVE computes grad and the two loss reductions.
"""

import numpy as np

C = 1000          # num classes
D = 2048          # feature dim
B = 8192          # batch (only used for the final divide; actual rows taken
                  # from the input)
NCORES = 8
P = 128           # partitions
CLS_PER = C // NCORES   # 125 classes per core
NSLICE = D // 512       # PSUM-bank-sized matmul column slices

# Static row capacities (per core). Smallest one >= max shard size is used so
# the compiled NEFF is stable across typical (uniform) inputs.
_CAPS = (1152, 1280, 1536, 2048, 4096, 8192)

# Set by test harness to collect a profile; harmless when False.
PROFILE = False
PROFILE_DIR = None
LAST_RESULT = None

_build_cache = {}


def _build(nt):
    """Build + compile the per-core Bass program for nt row-tiles of 128."""
    import concourse.bacc as bacc
    import concourse.bass as bass
    import concourse.mybir as mybir
    import concourse.tile as tile

    f32 = mybir.dt.float32
    f16 = mybir.dt.float16
    Alu = mybir.AluOpType

    nc = bacc.Bacc("TRN2", target_bir_lowering=False, debug=False,
                   num_devices=NCORES)


    # feat travels as fp16 (host converts): halves the dominant DMA stream
    # and feeds the PE at 1 row/cycle. centers/grad stay f32 — grad accuracy
    # is set by centers and the fp32 PSUM accumulation of the one-hot matmul.
    feat_d = nc.dram_tensor("feat", [nt, P, D], f16, kind="ExternalInput")
    ycls_d = nc.dram_tensor("ycls", [P, nt], f32, kind="ExternalInput")
    iota_d = nc.dram_tensor("iota", [P, P], f32, kind="ExternalInput")
    cen_d = nc.dram_tensor("cen", [P, D], f32, kind="ExternalInput")
    ab_d = nc.dram_tensor("ab", [P, 2], f32, kind="ExternalInput")  # [a, -b]
    grad_d = nc.dram_tensor("grad", [P, D], f32, kind="ExternalOutput")
    # stats columns: [0, nt) = per-tile sum(f^2); nt = ||c_c||^2;
    # nt+1 = <sums_c, c_c>
    stats_d = nc.dram_tensor("stats", [P, nt + 2], f32,
                             kind="ExternalOutput")

    with tile.TileContext(nc) as tc:
        with (
            tc.tile_pool(name="const", bufs=1) as constp,
            tc.tile_pool(name="feat", bufs=6) as featp,
            tc.tile_pool(name="small", bufs=6) as smallp,
            tc.tile_pool(name="scratch", bufs=2) as scrp,
            tc.tile_pool(name="outs", bufs=1) as outp,
            tc.tile_pool(name="psum", bufs=1, space=bass.MemorySpace.PSUM) as psump,
        ):
            sums_ps = psump.tile([P, D], f32)     # 4 PSUM banks, [class, D]
            stats_t = outp.tile([P, nt + 2], f32)

            # DMA order: first feat tile ahead of the tiny label/iota loads
            # (the small transfers overlap the big one on other queues), then
            # the bulk stream; centers land at ~2/3 of the stream.
            ys_t = constp.tile([P, nt], f32)
            iota_t = constp.tile([P, P], f32)
            fts = []
            cen_t = constp.tile([P, D], f32)
            ab_t = constp.tile([P, 2], f32)
            for t in range(nt):
                ft = featp.tile([P, D], f16, tag="ft")
                nc.sync.dma_start(ft[:], feat_d[t])
                fts.append(ft)
                if t == 0:
                    nc.sync.dma_start(ys_t[:], ycls_d[:])
                    nc.sync.dma_start(iota_t[:], iota_d[:])
                elif t == 6:
                    # centers + coeffs late in the feat stream: early enough
                    # for ScalarE to run t1/normsq before the tail, late
                    # enough not to starve the PE while it is still cold.
                    nc.sync.dma_start(cen_t[:], cen_d[:])
                    nc.sync.dma_start(ab_t[:], ab_d[:])

            # Alternate squares ScalarE/VectorE so they run concurrently;
            # ACT takes the last tile so DVE is free when the tail starts.
            act_sq = {0, 2, 4, 6, 8, 10, 12, 14, 16}
            t1 = outp.tile([P, D], f32, tag="t1")
            junk_n = outp.tile([P, D], f32, tag="junk_n")
            for t in range(nt):
                if t == 8:
                    # a*centers and ||c_c||^2 hoisted into ScalarE's idle
                    # window mid-stream; the tail's grad slices need t1.
                    nc.scalar.mul(t1[:], cen_t[:], ab_t[:, 0:1])
                    nc.scalar.activation(
                        junk_n[:], cen_t[:],
                        mybir.ActivationFunctionType.Square,
                        accum_out=stats_t[:, nt:nt + 1])
                ft = fts[t]
                oh = smallp.tile([P, P], f16, tag="oh")
                nc.vector.tensor_scalar(oh[:], iota_t[:],
                                        ys_t[:, t:t + 1], None,
                                        Alu.is_equal)
                for n in range(NSLICE):
                    nc.tensor.matmul(
                        sums_ps[:, n * 512:(n + 1) * 512],
                        oh[:],
                        ft[:, n * 512:(n + 1) * 512],
                        start=(t == 0),
                        stop=(t == nt - 1),
                    )

                sq_scr = scrp.tile([P, D], f16, tag="sqscr")
                acc = stats_t[:, t:t + 1]
                if t in act_sq:
                    nc.scalar.activation(
                        sq_scr[:], ft[:],
                        mybir.ActivationFunctionType.Square,
                        accum_out=acc)
                else:
                    nc.vector.scalar_tensor_tensor(
                        sq_scr[:], ft[:], 1.0, ft[:],
                        Alu.mult, Alu.mult, accum_out=acc)

            # Tail: per 512-column slice, grad = (-b)*sums + a*cen on DVE,
            # DMA it out immediately; the <sums_c, c_c> reduction follows.
            for n in range(NSLICE):
                sl = slice(n * 512, (n + 1) * 512)
                g_n = outp.tile([P, 512], f32, tag=f"grad{n}")
                nc.vector.scalar_tensor_tensor(g_n[:], sums_ps[:, sl],
                                               ab_t[:, 1:2], t1[:, sl],
                                               Alu.mult, Alu.add)
                nc.sync.dma_start(grad_d[:, sl], g_n[:])
            junk2 = outp.tile([P, D], f32, tag="junk2")
            nc.vector.scalar_tensor_tensor(
                junk2[:], sums_ps[:], 1.0, cen_t[:], Alu.mult, Alu.mult,
                accum_out=stats_t[:, nt + 1:nt + 2])
            nc.sync.dma_start(stats_d[:], stats_t[:])

    nc.compile()
    if not nc.is_finalized():
        nc.finalize()
    return nc


def _prepare(y, deep_feat, centers):
    """Host-side sharding: sort rows by class, build per-core input maps."""
    y = np.asarray(y)
    feat = np.ascontiguousarray(np.asarray(deep_feat), dtype=np.float32)
    cen = np.ascontiguousarray(np.asarray(centers), dtype=np.float32)

    yi = y.astype(np.int64)
    order = np.argsort(yi, kind="stable")
    ysorted = yi[order]
    cnt = np.bincount(ysorted, minlength=C).astype(np.int64)

    # Row range owned by each core (classes [125g, 125(g+1)))
    bounds = np.searchsorted(ysorted, np.arange(NCORES + 1) * CLS_PER)
    shard_rows = np.diff(bounds)
    cap = next((c for c in _CAPS if c >= shard_rows.max()), None)
    if cap is None:
        raise ValueError(f"shard of {shard_rows.max()} rows exceeds max capacity")
    nt = cap // P

    iota = np.broadcast_to(np.arange(P, dtype=np.float32), (P, P)).copy()

    in_maps = []
    host_cnt = []   # per-core padded local counts, for the loss combine
    for g in range(NCORES):
        lo, hi = bounds[g], bounds[g + 1]
        n_g = hi - lo

        feat_g = np.zeros((cap, D), dtype=np.float16)
        feat_g[:n_g] = feat[order[lo:hi]].astype(np.float16)

        ycls_g = np.full((cap,), CLS_PER + 1, dtype=np.float32)  # padding=126
        ycls_g[:n_g] = (ysorted[lo:hi] - g * CLS_PER).astype(np.float32)

        cen_g = np.zeros((P, D), dtype=np.float32)
        cen_g[:CLS_PER] = cen[g * CLS_PER:(g + 1) * CLS_PER]

        cnt_g = np.zeros((P,), dtype=np.float64)
        cnt_g[:CLS_PER] = cnt[g * CLS_PER:(g + 1) * CLS_PER]
        host_cnt.append(cnt_g)

        a_g = (cnt_g / (1.0 + cnt_g)).astype(np.float32)
        bn_g = np.where(cnt_g > 0, -1.0 / (1.0 + cnt_g), 0.0).astype(np.float32)

        in_maps.append({
            "feat": feat_g.reshape(nt, P, D),
            "ycls": np.ascontiguousarray(ycls_g.reshape(nt, P).T),
            "iota": iota,
            "cen": cen_g,
            "ab": np.stack([a_g, bn_g], axis=1),
        })

    return in_maps, host_cnt, nt


def _combine(results, host_cnt, nt, nrows):
    """Assemble full grad + loss from per-core outputs."""
    grad_full = np.concatenate(
        [results[g]["grad"][:CLS_PER] for g in range(NCORES)], axis=0)

    total = 0.0
    for g in range(NCORES):
        st = results[g]["stats"].astype(np.float64)
        sq = st[:, :nt].sum()
        normsq = st[:, nt]
        dotsc = st[:, nt + 1].sum()
        total += sq + (host_cnt[g] * normsq).sum() - 2.0 * dotsc

    loss = np.float32(0.5 * np.sqrt(total) / nrows)
    return loss, grad_full


def kernel(y, deep_feat, centers):
    global LAST_RESULT
    from concourse.bass_utils import run_bass_kernel_spmd

    nrows = np.asarray(deep_feat).shape[0]
    in_maps, host_cnt, nt = _prepare(y, deep_feat, centers)

    if nt not in _build_cache:
        _build_cache[nt] = _build(nt)
    nc = _build_cache[nt]

    res = run_bass_kernel_spmd(nc, in_maps, list(range(NCORES)),
                               trace=PROFILE, tmpdir=PROFILE_DIR)
    LAST_RESULT = res
    return _combine(res.results, host_cnt, nt, nrows)
